# revision 33
# baseline (speedup 1.0000x reference)
"""Trainium2 Bass kernel for causal attention with relative-position bias.

Problem (hardcoded): B=16 heads, S=2048, Dh=64, fp32 I/O.
  dots = Q@K^T; bias pos=Q@R_w^T+R_b gathered by sign(j-i)+1; causal mask
  (-1e10 above diag); softmax(dots/sqrt(512)); out = probs@V.

Algebra: within row q the gathered bias is a constant pos0[q] for k<q and
pos1[q] at k==q (k>q masked). Softmax is invariant to per-row constants, so
only the diagonal needs exp((Q[q].(K[q]+R_w[1]-R_w[0]) + R_b[1]-R_b[0])/s).
Logits are small (|z|<=~2.2) so exp runs without max subtraction and the
masked entries are exact zeros.

Layout: scores computed transposed, S^T[k,q] (k on partitions):
  S^T = (K^T chunk).T @ Q^T      (lhsT=K^T[64,128], rhs=Q^T[64,ncols])
  out^T[d,q]+denominator row = [V|1].T @ exp(S^T)  (accumulated over chunks)
Q^T/K^T come from tensor-engine transposes staged through the score-PSUM
rotation (fp16 bitcast view); the [65,S] result is cast fp16, xbar-DMA
transposed back to natural layout, and divided by the denominator row.

Scheduling: per 1024-col q-phase, fills flow QK(PE) -> exp(ACT) -> PV(PE)
with QK emitted two fills ahead (3 PSUM score buffers), the PV of the
diagonal 128 cols split off (it alone waits on the DVE mask/pred ops), and
junk matmuls keeping the PE HAM clock at 8/8.

Sharding: 16 heads -> 8 NeuronCores, 2 heads/core, no communication.
"""

import os
import sys

if "/opt/trn_rl_repo" not in sys.path:
    sys.path.insert(0, "/opt/trn_rl_repo")

import numpy as np

import concourse.bacc as bacc
import concourse.mybir as mybir
import concourse.tile as tile
from concourse.bass_utils import run_bass_kernel_spmd
from concourse.masks import make_identity, make_upper_triangular

B, S, DH = 16, 2048, 64
N_CORES = 8
HPC = B // N_CORES  # heads per core
P = 128
NT = S // P  # 16 q/k tiles per head
VW = 66  # V row width in SBUF: 64 values + ones col + pad (66*2B keeps 4B align)
OW = 80  # out^T rows padded to xbar multiple of 16 (64 vals + denom + 15 pad)
PH = 1024  # q-phase width
INV_SCALE = float(1.0 / np.sqrt(np.float32(512.0)))

f16 = mybir.dt.float16
f32 = mybir.dt.float32


def _emit(ctx, tc, q_d, k_d, v_d, rw_d, rb_d, out_d):
    nc = tc.nc
    AF = mybir.ActivationFunctionType

    const = ctx.enter_context(tc.tile_pool(name="const", bufs=1))
    ld = ctx.enter_context(tc.tile_pool(name="ld", bufs=2))
    hp = ctx.enter_context(tc.tile_pool(name="hp", bufs=2))
    slabp = ctx.enter_context(tc.tile_pool(name="slab", bufs=5))
    outp = ctx.enter_context(tc.tile_pool(name="outp", bufs=2))
    psc = ctx.enter_context(tc.tile_pool(name="psc", bufs=3, space="PSUM"))
    pout = ctx.enter_context(tc.tile_pool(name="pout", bufs=1, space="PSUM"))

    # constants ----------------------------------------------------------
    m01 = const.tile([P, P], f16)  # 1.0 strictly above diagonal (valid k<q)
    make_upper_triangular(nc, m01[:], val=1.0, diag=False)
    id01 = const.tile([P, P], mybir.dt.int8)
    make_identity(nc, id01[:])
    idm = const.tile([P, P], f16)  # numeric identity for PE-mode transpose
    make_identity(nc, idm[:])

    # broadcast R_w rows 0/1 and R_b[0:2] to all partitions (0-step DMA reads)
    rbc = const.tile([P, 2 * DH + 2], f32)
    nc.gpsimd.dma_start(out=rbc[:, 0:DH], in_=rw_d[0:1, :].partition_broadcast(P))
    nc.gpsimd.dma_start(out=rbc[:, DH : 2 * DH], in_=rw_d[1:2, :].partition_broadcast(P))
    nc.gpsimd.dma_start(
        out=rbc[:, 2 * DH : 2 * DH + 2], in_=rb_d[None, 0:2].partition_broadcast(P)
    )
    rd16 = const.tile([P, DH], f16)  # R_w[1]-R_w[0], fp16, bcast on partitions
    nc.vector.tensor_sub(rd16[:], rbc[:, DH : 2 * DH], rbc[:, 0:DH])
    rbbias = const.tile([P, 1], f32)  # (R_b[1]-R_b[0]) / scale
    nc.vector.tensor_sub(
        rbbias[:], rbc[:, 2 * DH + 1 : 2 * DH + 2], rbc[:, 2 * DH : 2 * DH + 1]
    )
    nc.vector.tensor_scalar_mul(rbbias[:], rbbias[:], INV_SCALE)

    # PE warm-up scaffolding: junk matmuls hold the HAM clock gate at 8/8.
    # Junk targets are PSUM tiles whose data is about to be overwritten
    # (start=True only resets has_written bits; readers never see junk).
    junk = const.tile([P, 512], f16)
    nc.gpsimd.memset(junk[:], 0.0)

    def junk_into(t, count=1, n=P):
        for _ in range(count):
            nc.tensor.matmul(
                t[0:32, 0:n], lhsT=junk[:, 0:32], rhs=junk[:, 0:n], start=True,
                stop=True, skip_group_check=True,
            )

    def junk_burst(t, count, n=512):
        # sustained burst long enough (~4us cold) to force HAM back to 8/8
        for _ in range(count):
            nc.tensor.matmul(
                t[0:32, 0:n], lhsT=junk[:, 0:32], rhs=junk[:, 0:n], start=True,
                stop=True, skip_group_check=True,
            )

    # startup warm-up burst (~5us of back-to-back matmuls while DMAs load)
    warm0 = psc.tile([P, PH], f32, tag="sc")
    for _ in range(10):
        nc.tensor.matmul(
            warm0[:, 0:512], lhsT=junk[:, 0:P], rhs=junk[:], start=True,
            stop=True, skip_group_check=True,
        )

    for h in range(HPC):
        # load + cast to fp16 -------------------------------------------
        q32 = ld.tile([P, NT * DH], f32, tag="ld32")
        nc.sync.dma_start(
            out=q32[:].rearrange("p (n d) -> p n d", d=DH),
            in_=q_d[h].rearrange("(n p) d -> p n d", p=P),
        )
        qf = hp.tile([P, NT * DH], f16, tag="qf")
        nc.vector.tensor_copy(qf[:], q32[:])

        k32 = ld.tile([P, NT * DH], f32, tag="ld32")
        nc.sync.dma_start(
            out=k32[:].rearrange("p (n d) -> p n d", d=DH),
            in_=k_d[h].rearrange("(n p) d -> p n d", p=P),
        )
        kf = hp.tile([P, NT * DH], f16, tag="kf")
        nc.vector.tensor_copy(kf[:], k32[:])

        v32 = ld.tile([P, NT * DH], f32, tag="ld32")
        nc.sync.dma_start(
            out=v32[:].rearrange("p (n d) -> p n d", d=DH),
            in_=v_d[h].rearrange("(n p) d -> p n d", p=P),
        )
        vaug = hp.tile([P, NT * VW], f16, tag="vaug")
        v3 = vaug[:].rearrange("p (n e) -> p n e", e=VW)
        nc.gpsimd.tensor_copy(
            v3[:, :, 0:DH], v32[:].rearrange("p (n d) -> p n d", d=DH)
        )
        nc.gpsimd.memset(v3[:, :, DH : DH + 1], 1.0)

        # transpose Q, K to [64, S] on the tensor engine, staged through a
        # score-rotation PSUM tile viewed as fp16 (no extra banks).
        def transpose_to(src, tag):
            dst = hp.tile([DH, S], f16, tag=tag)
            tp = psc.tile([P, PH], f32, tag="sc")
            tp16 = tp[:].bitcast(f16)  # [128, 2048] fp16 view
            s3 = src[:].rearrange("p (n d) -> p n d", d=DH)
            for t in range(NT):
                nc.tensor.transpose(
                    tp16[0:DH, t * P : (t + 1) * P], s3[:, t, :], idm[:]
                )
                if h == 0 and t % 3 == 1:
                    junk_into(warm0)
            nc.vector.tensor_copy(dst[:], tp16[0:DH, :])
            return dst

        qt = transpose_to(qf, "qt")
        kt = transpose_to(kf, "kt")

        # diagonal terms: pre[q] = Q[q] . (K[q] + rdelta) ---------------
        t2 = ld.tile([P, NT * DH], f16, tag="t2")
        t2_3 = t2[:].rearrange("p (n d) -> p n d", d=DH)
        nc.vector.tensor_add(
            t2_3,
            kf[:].rearrange("p (n d) -> p n d", d=DH),
            rd16[:, None, :].to_broadcast([P, NT, DH]),
        )
        nc.vector.tensor_mul(t2[:], qf[:], t2[:])
        pre = hp.tile([P, NT], f32, tag="pre")
        nc.vector.tensor_reduce(
            out=pre[:], in_=t2_3, axis=mybir.AxisListType.X, op=mybir.AluOpType.add
        )
        pdiag = hp.tile([P, NT], f16, tag="pdiag")
        nc.scalar.activation(
            pdiag[:], pre[:], AF.Exp, bias=rbbias[:, 0:1], scale=INV_SCALE
        )

        # main loop ------------------------------------------------------
        outTs = outp.tile([OW, S], f16, tag="outTs")
        nc.gpsimd.memset(outTs[DH : OW, :], 0.0)
        for ph in range(S // PH):
            lo, hi = ph * PH, (ph + 1) * PH
            fills = []
            for ki in range(NT):
                q0 = P * ki
                base = max(q0, lo)
                if base < hi:
                    fills.append((ki, q0, base, hi - base))
            outT = pout.tile([DH + 1, PH], f32, tag="outT")

            def emit_qk(f):
                ki, q0, base, n = fills[f]
                sc = psc.tile([P, PH], f32, tag="sc")
                junk_burst(sc, 12) if f == 0 else junk_into(
                    sc, count=(4 if f == 1 else 1), n=192
                )
                for so in range(0, n, 512):
                    nn = min(512, n - so)
                    nc.tensor.matmul(
                        sc[:, so : so + nn],
                        lhsT=kt[:, q0 : q0 + P],
                        rhs=qt[:, base + so : base + so + nn],
                        start=True,
                        stop=True,
                    )
                return sc

            last_ki = fills[-1][0]

            def emit_pv(f, slab):
                ki, q0, base, n = fills[f]
                # PV: diagonal 128 cols go in a separate trailing matmul so
                # the rest of the fill only waits on the exp.
                segs = []
                for qb in range(base // 512, (base + n - 1) // 512 + 1):
                    g0 = max(base, qb * 512)
                    g1 = min(base + n, (qb + 1) * 512)
                    segs.append((g0, g1, ki == min(last_ki, 4 * qb + 3)))
                diag = None
                if base == q0:
                    g0, g1, stp = segs[0]
                    diag = (g0, stp if g1 - g0 <= P else False)
                    segs = ([] if g1 - g0 <= P else [(g0 + P, g1, stp)]) + segs[1:]
                for g0, g1, stp in segs:
                    nc.tensor.matmul(
                        outT[:, g0 - lo : g1 - lo],
                        lhsT=v3[:, ki, 0 : DH + 1],
                        rhs=slab[:, g0 - base : g1 - base],
                        start=(ki == 0),
                        stop=stp,
                        skip_group_check=True,
                    )
                if diag is not None:
                    g0, stp = diag
                    nc.tensor.matmul(
                        outT[:, g0 - lo : g0 - lo + P],
                        lhsT=v3[:, ki, 0 : DH + 1],
                        rhs=slab[:, 0:P],
                        start=False,
                        stop=stp,
                        skip_group_check=True,
                    )

            scs = {0: emit_qk(0)}
            if len(fills) > 1:
                scs[1] = emit_qk(1)
            pend = []  # PV runs one fill behind its exp to lengthen the ring
            for f, (ki, q0, base, n) in enumerate(fills):
                sc = scs.pop(f)
                slab = slabp.tile([P, PH], f16, tag="slab")
                nc.scalar.activation(
                    slab[:, 0:n], sc[:, 0:n], AF.Exp, scale=INV_SCALE
                )
                if base == q0:
                    # diagonal 128x128 block: zero k>=q, then write exp diag
                    nc.vector.tensor_mul(slab[:, 0:P], slab[:, 0:P], m01[:])
                    nc.vector.copy_predicated(
                        slab[:, 0:P], id01[:],
                        pdiag[:, ki : ki + 1].to_broadcast([P, P]),
                    )
                if f + 2 < len(fills):
                    scs[f + 2] = emit_qk(f + 2)
                if pend:
                    emit_pv(*pend.pop(0))
                pend.append((f, slab))
            while pend:
                emit_pv(*pend.pop(0))
            nc.vector.tensor_copy(outTs[0 : DH + 1, lo:hi], outT[:, :])

            # per-phase epilogue: transpose back, divide, store the 8
            # q-tiles of this phase while the next phase computes.
            NP = PH // P  # q-tiles per phase
            onat = outp.tile([P, NP * OW], f16, tag="onat")
            onat3 = onat[:].rearrange("p (n e) -> p n e", e=OW)
            nc.sync.dma_start_transpose(out=onat3, in_=outTs[:, lo:hi])
            recip = outp.tile([P, NP], f32, tag="recip")
            nc.vector.reciprocal(recip[:, :, None], onat3[:, :, DH : DH + 1])
            ofin = outp.tile([P, NP * DH], f32, tag="ofin")
            nc.vector.tensor_mul(
                ofin[:].rearrange("p (n d) -> p n d", d=DH),
                onat3[:, :, 0:DH],
                recip[:, :, None].to_broadcast([P, NP, DH]),
            )
            nc.sync.dma_start(
                out=out_d[h].rearrange("(n p) d -> p n d", p=P)[
                    :, ph * NP : (ph + 1) * NP, :
                ],
                in_=ofin[:].rearrange("p (n d) -> p n d", d=DH),
            )


def build_nc(debug=False):
    from contextlib import ExitStack

    nc = bacc.Bacc("TRN2", target_bir_lowering=False, debug=debug, num_devices=N_CORES)
    q_d = nc.dram_tensor("query", [HPC, S, DH], f32, kind="ExternalInput").ap()
    k_d = nc.dram_tensor("key", [HPC, S, DH], f32, kind="ExternalInput").ap()
    v_d = nc.dram_tensor("value", [HPC, S, DH], f32, kind="ExternalInput").ap()
    rw_d = nc.dram_tensor("R_w", [3, DH], f32, kind="ExternalInput").ap()
    rb_d = nc.dram_tensor("R_b", [3], f32, kind="ExternalInput").ap()
    out_d = nc.dram_tensor("out", [HPC, S, DH], f32, kind="ExternalOutput").ap()
    with tile.TileContext(nc) as tc, ExitStack() as ctx:
        _emit(ctx, tc, q_d, k_d, v_d, rw_d, rb_d, out_d)
    nc.finalize()
    return nc


_NC_CACHE = {}


def _get_nc():
    if "nc" not in _NC_CACHE:
        _NC_CACHE["nc"] = build_nc()
    return _NC_CACHE["nc"]


def kernel(query, key, value, R_w, R_b, trace=False):
    query = np.ascontiguousarray(np.asarray(query, dtype=np.float32))
    key = np.ascontiguousarray(np.asarray(key, dtype=np.float32))
    value = np.ascontiguousarray(np.asarray(value, dtype=np.float32))
    R_w = np.ascontiguousarray(np.asarray(R_w, dtype=np.float32))
    R_b = np.ascontiguousarray(np.asarray(R_b, dtype=np.float32))

    nc = _get_nc()
    in_maps = [
        {
            "query": query[c * HPC : (c + 1) * HPC],
            "key": key[c * HPC : (c + 1) * HPC],
            "value": value[c * HPC : (c + 1) * HPC],
            "R_w": R_w,
            "R_b": R_b,
        }
        for c in range(N_CORES)
    ]
    res = run_bass_kernel_spmd(nc, in_maps, core_ids=list(range(N_CORES)), trace=trace)
    out = np.concatenate([res.results[c]["out"] for c in range(N_CORES)], axis=0)
    if trace:
        kernel.last_results = res
    return out.astype(np.float32, copy=False)


# revision 34
# speedup vs baseline: 1.0809x; 1.0809x over previous
"""Trainium2 Bass kernel for causal attention with relative-position bias.

Problem (hardcoded): B=16 heads, S=2048, Dh=64, fp32 I/O.
  dots = Q@K^T; bias pos=Q@R_w^T+R_b gathered by sign(j-i)+1; causal mask
  (-1e10 above diag); softmax(dots/sqrt(512)); out = probs@V.

Algebra: within row q the gathered bias is a constant pos0[q] for k<q and
pos1[q] at k==q (k>q masked). Softmax is invariant to per-row constants, so
only the diagonal needs exp((Q[q].(K[q]+R_w[1]-R_w[0]) + R_b[1]-R_b[0])/s).
Logits are small (|z|<=~2.2) so exp runs without max subtraction and the
masked entries are exact zeros.

Layout: scores computed transposed, S^T[k,q] (k on partitions):
  S^T = (K^T chunk).T @ Q^T      (lhsT=K^T[64,128], rhs=Q^T[64,ncols])
  out^T[d,q]+denominator row = [V|1].T @ exp(S^T)  (accumulated over chunks)
Q^T/K^T come from tensor-engine transposes staged through the score-PSUM
rotation (fp16 bitcast view); the [65,S] result is cast fp16, xbar-DMA
transposed back to natural layout, and divided by the denominator row.

Scheduling: per 1024-col q-phase, fills flow QK(PE) -> exp(ACT) -> PV(PE)
with QK emitted two fills ahead (3 PSUM score buffers), the PV of the
diagonal 128 cols split off (it alone waits on the DVE mask/pred ops), and
junk matmuls keeping the PE HAM clock at 8/8.

Sharding: 16 heads -> 8 NeuronCores, 2 heads/core, no communication.
"""

import os
import sys

if "/opt/trn_rl_repo" not in sys.path:
    sys.path.insert(0, "/opt/trn_rl_repo")

import numpy as np

import concourse.bacc as bacc
import concourse.mybir as mybir
import concourse.tile as tile
from concourse.bass_utils import run_bass_kernel_spmd
from concourse.masks import make_identity, make_upper_triangular

B, S, DH = 16, 2048, 64
N_CORES = 8
HPC = B // N_CORES  # heads per core
P = 128
NT = S // P  # 16 q/k tiles per head
VW = 66  # V row width in SBUF: 64 values + ones col + pad (66*2B keeps 4B align)
OW = 80  # out^T rows padded to xbar multiple of 16 (64 vals + denom + 15 pad)
PH = 1024  # q-phase width
INV_SCALE = float(1.0 / np.sqrt(np.float32(512.0)))

f16 = mybir.dt.float16
f32 = mybir.dt.float32


def _emit(ctx, tc, q_d, k_d, v_d, rw_d, rb_d, out_d):
    nc = tc.nc
    AF = mybir.ActivationFunctionType

    const = ctx.enter_context(tc.tile_pool(name="const", bufs=1))
    ld = ctx.enter_context(tc.tile_pool(name="ld", bufs=2))
    hp = ctx.enter_context(tc.tile_pool(name="hp", bufs=2))
    slabp = ctx.enter_context(tc.tile_pool(name="slab", bufs=5))
    outp = ctx.enter_context(tc.tile_pool(name="outp", bufs=2))
    psc = ctx.enter_context(tc.tile_pool(name="psc", bufs=3, space="PSUM"))
    pout = ctx.enter_context(tc.tile_pool(name="pout", bufs=1, space="PSUM"))

    # constants ----------------------------------------------------------
    m01 = const.tile([P, P], f16)  # 1.0 strictly above diagonal (valid k<q)
    make_upper_triangular(nc, m01[:], val=1.0, diag=False)
    id01 = const.tile([P, P], mybir.dt.int8)
    make_identity(nc, id01[:])
    idm = const.tile([P, P], f16)  # numeric identity for PE-mode transpose
    make_identity(nc, idm[:])

    # broadcast R_w rows 0/1 and R_b[0:2] to all partitions (0-step DMA reads)
    rbc = const.tile([P, 2 * DH + 2], f32)
    nc.gpsimd.dma_start(out=rbc[:, 0:DH], in_=rw_d[0:1, :].partition_broadcast(P))
    nc.gpsimd.dma_start(out=rbc[:, DH : 2 * DH], in_=rw_d[1:2, :].partition_broadcast(P))
    nc.gpsimd.dma_start(
        out=rbc[:, 2 * DH : 2 * DH + 2], in_=rb_d[None, 0:2].partition_broadcast(P)
    )
    rd16 = const.tile([P, DH], f16)  # R_w[1]-R_w[0], fp16, bcast on partitions
    nc.vector.tensor_sub(rd16[:], rbc[:, DH : 2 * DH], rbc[:, 0:DH])
    rbbias = const.tile([P, 1], f32)  # (R_b[1]-R_b[0]) / scale
    nc.vector.tensor_sub(
        rbbias[:], rbc[:, 2 * DH + 1 : 2 * DH + 2], rbc[:, 2 * DH : 2 * DH + 1]
    )
    nc.vector.tensor_scalar_mul(rbbias[:], rbbias[:], INV_SCALE)

    # PE warm-up scaffolding: junk matmuls hold the HAM clock gate at 8/8.
    # Junk targets are PSUM tiles whose data is about to be overwritten
    # (start=True only resets has_written bits; readers never see junk).
    junk = const.tile([P, 512], f16)
    nc.gpsimd.memset(junk[:], 0.0)

    def junk_into(t, count=1, n=P):
        for _ in range(count):
            nc.tensor.matmul(
                t[0:32, 0:n], lhsT=junk[:, 0:32], rhs=junk[:, 0:n], start=True,
                stop=True, skip_group_check=True,
            )

    def junk_burst(t, count, n=512):
        # sustained burst long enough (~4us cold) to force HAM back to 8/8
        for _ in range(count):
            nc.tensor.matmul(
                t[0:32, 0:n], lhsT=junk[:, 0:32], rhs=junk[:, 0:n], start=True,
                stop=True, skip_group_check=True,
            )

    # startup warm-up burst (~5us of back-to-back matmuls while DMAs load)
    warm0 = psc.tile([P, PH], f32, tag="sc")
    for _ in range(10):
        nc.tensor.matmul(
            warm0[:, 0:512], lhsT=junk[:, 0:P], rhs=junk[:], start=True,
            stop=True, skip_group_check=True,
        )

    for h in range(HPC):
        # load + cast to fp16 -------------------------------------------
        q32 = ld.tile([P, NT * DH], f32, tag="ld32")
        nc.sync.dma_start(
            out=q32[:].rearrange("p (n d) -> p n d", d=DH),
            in_=q_d[h].rearrange("(n p) d -> p n d", p=P),
        )
        qf = hp.tile([P, NT * DH], f16, tag="qf")
        nc.vector.tensor_copy(qf[:], q32[:])

        k32 = ld.tile([P, NT * DH], f32, tag="ld32")
        nc.sync.dma_start(
            out=k32[:].rearrange("p (n d) -> p n d", d=DH),
            in_=k_d[h].rearrange("(n p) d -> p n d", p=P),
        )
        kf = hp.tile([P, NT * DH], f16, tag="kf")
        nc.vector.tensor_copy(kf[:], k32[:])

        v32 = ld.tile([P, NT * DH], f32, tag="ld32")
        nc.sync.dma_start(
            out=v32[:].rearrange("p (n d) -> p n d", d=DH),
            in_=v_d[h].rearrange("(n p) d -> p n d", p=P),
        )
        vaug = hp.tile([P, NT * VW], f16, tag="vaug")
        v3 = vaug[:].rearrange("p (n e) -> p n e", e=VW)
        nc.gpsimd.tensor_copy(
            v3[:, :, 0:DH], v32[:].rearrange("p (n d) -> p n d", d=DH)
        )
        nc.gpsimd.memset(v3[:, :, DH : DH + 1], 1.0)

        # transpose Q, K to [64, S] on the tensor engine, staged through a
        # score-rotation PSUM tile viewed as fp16 (no extra banks).
        def transpose_to(src, tag):
            dst = hp.tile([DH, S], f16, tag=tag)
            tp = psc.tile([P, PH], f32, tag="sc")
            tp16 = tp[:].bitcast(f16)  # [128, 2048] fp16 view
            s3 = src[:].rearrange("p (n d) -> p n d", d=DH)
            for t in range(NT):
                nc.tensor.transpose(
                    tp16[0:DH, t * P : (t + 1) * P], s3[:, t, :], idm[:]
                )
                if h == 0 and t % 3 == 1:
                    junk_into(warm0)
            nc.vector.tensor_copy(dst[:, 0 : S // 2], tp16[0:DH, 0 : S // 2])
            nc.vector.tensor_copy(dst[:, S // 2 : S], tp16[0:DH, S // 2 : S])
            return dst

        qt = transpose_to(qf, "qt")
        kt = transpose_to(kf, "kt")

        # diagonal terms: pre[q] = Q[q] . (K[q] + rdelta) ---------------
        t2 = ld.tile([P, NT * DH], f16, tag="t2")
        t2_3 = t2[:].rearrange("p (n d) -> p n d", d=DH)
        nc.vector.tensor_add(
            t2_3,
            kf[:].rearrange("p (n d) -> p n d", d=DH),
            rd16[:, None, :].to_broadcast([P, NT, DH]),
        )
        nc.vector.tensor_mul(t2[:], qf[:], t2[:])
        pre = hp.tile([P, NT], f32, tag="pre")
        nc.vector.tensor_reduce(
            out=pre[:], in_=t2_3, axis=mybir.AxisListType.X, op=mybir.AluOpType.add
        )
        pdiag = hp.tile([P, NT], f16, tag="pdiag")
        pdiag_emitted = [False]

        def emit_pdiag():
            if not pdiag_emitted[0]:
                pdiag_emitted[0] = True
                nc.scalar.activation(
                    pdiag[:], pre[:], AF.Exp, bias=rbbias[:, 0:1], scale=INV_SCALE
                )

        # main loop ------------------------------------------------------
        outTs = outp.tile([OW, S], f16, tag="outTs")
        nc.gpsimd.memset(outTs[DH : OW, :], 0.0)
        for ph in range(S // PH):
            lo, hi = ph * PH, (ph + 1) * PH
            fills = []
            for ki in range(NT):
                q0 = P * ki
                base = max(q0, lo)
                if base < hi:
                    fills.append((ki, q0, base, hi - base))
            outT = pout.tile([DH + 1, PH], f32, tag="outT")

            def emit_qk(f):
                ki, q0, base, n = fills[f]
                sc = psc.tile([P, PH], f32, tag="sc")
                junk_burst(sc, 12) if f == 0 else junk_into(
                    sc, count=(4 if f == 1 else 1), n=192
                )
                for so in range(0, n, 512):
                    nn = min(512, n - so)
                    nc.tensor.matmul(
                        sc[:, so : so + nn],
                        lhsT=kt[:, q0 : q0 + P],
                        rhs=qt[:, base + so : base + so + nn],
                        start=True,
                        stop=True,
                    )
                return sc

            last_ki = fills[-1][0]

            def emit_pv(f, slab):
                ki, q0, base, n = fills[f]
                # PV: diagonal 128 cols go in a separate trailing matmul so
                # the rest of the fill only waits on the exp.
                segs = []
                for qb in range(base // 512, (base + n - 1) // 512 + 1):
                    g0 = max(base, qb * 512)
                    g1 = min(base + n, (qb + 1) * 512)
                    segs.append((g0, g1, ki == min(last_ki, 4 * qb + 3)))
                diag = None
                if base == q0:
                    g0, g1, stp = segs[0]
                    diag = (g0, stp if g1 - g0 <= P else False)
                    segs = ([] if g1 - g0 <= P else [(g0 + P, g1, stp)]) + segs[1:]
                for g0, g1, stp in segs:
                    nc.tensor.matmul(
                        outT[:, g0 - lo : g1 - lo],
                        lhsT=v3[:, ki, 0 : DH + 1],
                        rhs=slab[:, g0 - base : g1 - base],
                        start=(ki == 0),
                        stop=stp,
                        skip_group_check=True,
                    )
                if diag is not None:
                    g0, stp = diag
                    nc.tensor.matmul(
                        outT[:, g0 - lo : g0 - lo + P],
                        lhsT=v3[:, ki, 0 : DH + 1],
                        rhs=slab[:, 0:P],
                        start=False,
                        stop=stp,
                        skip_group_check=True,
                    )

            scs = {0: emit_qk(0)}
            if len(fills) > 1:
                scs[1] = emit_qk(1)
            pend = []  # PV runs one fill behind its exp to lengthen the ring
            for f, (ki, q0, base, n) in enumerate(fills):
                sc = scs.pop(f)
                slab = slabp.tile([P, PH], f16, tag="slab")
                nc.scalar.activation(
                    slab[:, 0:n], sc[:, 0:n], AF.Exp, scale=INV_SCALE
                )
                if base == q0:
                    # diagonal 128x128 block: zero k>=q, then write exp diag
                    emit_pdiag()
                    nc.vector.tensor_mul(slab[:, 0:P], slab[:, 0:P], m01[:])
                    nc.vector.copy_predicated(
                        slab[:, 0:P], id01[:],
                        pdiag[:, ki : ki + 1].to_broadcast([P, P]),
                    )
                if f + 2 < len(fills):
                    scs[f + 2] = emit_qk(f + 2)
                if pend:
                    emit_pv(*pend.pop(0))
                pend.append((f, slab))
            while pend:
                emit_pv(*pend.pop(0))
            nc.vector.tensor_copy(outTs[0 : DH + 1, lo:hi], outT[:, :])

            # per-phase epilogue: transpose back, divide, store the 8
            # q-tiles of this phase while the next phase computes.
            NP = PH // P  # q-tiles per phase
            onat = outp.tile([P, NP * OW], f16, tag="onat")
            onat3 = onat[:].rearrange("p (n e) -> p n e", e=OW)
            nc.sync.dma_start_transpose(out=onat3, in_=outTs[:, lo:hi])
            recip = outp.tile([P, NP], f32, tag="recip")
            nc.vector.reciprocal(recip[:, :, None], onat3[:, :, DH : DH + 1])
            ofin = outp.tile([P, NP * DH], f32, tag="ofin")
            nc.vector.tensor_mul(
                ofin[:].rearrange("p (n d) -> p n d", d=DH),
                onat3[:, :, 0:DH],
                recip[:, :, None].to_broadcast([P, NP, DH]),
            )
            nc.sync.dma_start(
                out=out_d[h].rearrange("(n p) d -> p n d", p=P)[
                    :, ph * NP : (ph + 1) * NP, :
                ],
                in_=ofin[:].rearrange("p (n d) -> p n d", d=DH),
            )


def build_nc(debug=False):
    from contextlib import ExitStack

    nc = bacc.Bacc("TRN2", target_bir_lowering=False, debug=debug, num_devices=N_CORES)
    q_d = nc.dram_tensor("query", [HPC, S, DH], f32, kind="ExternalInput").ap()
    k_d = nc.dram_tensor("key", [HPC, S, DH], f32, kind="ExternalInput").ap()
    v_d = nc.dram_tensor("value", [HPC, S, DH], f32, kind="ExternalInput").ap()
    rw_d = nc.dram_tensor("R_w", [3, DH], f32, kind="ExternalInput").ap()
    rb_d = nc.dram_tensor("R_b", [3], f32, kind="ExternalInput").ap()
    out_d = nc.dram_tensor("out", [HPC, S, DH], f32, kind="ExternalOutput").ap()
    with tile.TileContext(nc) as tc, ExitStack() as ctx:
        _emit(ctx, tc, q_d, k_d, v_d, rw_d, rb_d, out_d)
    nc.finalize()
    return nc


_NC_CACHE = {}


def _get_nc():
    if "nc" not in _NC_CACHE:
        _NC_CACHE["nc"] = build_nc()
    return _NC_CACHE["nc"]


def kernel(query, key, value, R_w, R_b, trace=False):
    query = np.ascontiguousarray(np.asarray(query, dtype=np.float32))
    key = np.ascontiguousarray(np.asarray(key, dtype=np.float32))
    value = np.ascontiguousarray(np.asarray(value, dtype=np.float32))
    R_w = np.ascontiguousarray(np.asarray(R_w, dtype=np.float32))
    R_b = np.ascontiguousarray(np.asarray(R_b, dtype=np.float32))

    nc = _get_nc()
    in_maps = [
        {
            "query": query[c * HPC : (c + 1) * HPC],
            "key": key[c * HPC : (c + 1) * HPC],
            "value": value[c * HPC : (c + 1) * HPC],
            "R_w": R_w,
            "R_b": R_b,
        }
        for c in range(N_CORES)
    ]
    res = run_bass_kernel_spmd(nc, in_maps, core_ids=list(range(N_CORES)), trace=trace)
    out = np.concatenate([res.results[c]["out"] for c in range(N_CORES)], axis=0)
    if trace:
        kernel.last_results = res
    return out.astype(np.float32, copy=False)
